# revision 16
# baseline (speedup 1.0000x reference)
"""Trainium2 Bass kernel for causal MHA block (b=4, s=2048, E=1024, 16 heads).

Sharding: tensor-parallel over heads — 2 heads per core across 8 cores.
Each core computes Q^T/K^T (transposed layout, head-packed), V (natural
layout, ones-augmented), block-causal attention with softmax denominators
obtained for free from the ones column, and a partial out-projection over
its 128 embedding dims. Host sums the 8 partials and adds out_b.

v2 scheduling rewrite (the math is the baseline's, the issue order isn't):
  - software-pipelined attention: scores for key-tile i+1 are issued on PE
    before ctx matmuls of tile i, so PE never blocks on the ACT (exp)
    semaphore round-trip.
  - proj/out-proj matmuls are interleaved as fillers BETWEEN attention
    iterations (deadline-JIT: proj chunk 4b+j+1 runs inside attn chunk
    (b,j)), keeping PE continuously busy (no HAM re-throttle).
  - per-head causal-tri adds merged into one N=256 matmul; per-head denom
    broadcasts merged into one K=2 matmul.
  - Q and K accumulate into one 2-bank PSUM tile, drained by a single DVE
    op (adding bq to K is harmless: per-key-constant shifts cancel in
    softmax over keys).
  - PSUM->SBUF staging (out-proj, V transpose+mask) moved to GpSimd.
  - PSUM: scores [128,2,512]x2 (4 banks) + ctx [65,512]x2 (2) + qk
    [128,2,512]x1 (2) rotating with a 1-bank work pool... total 8 banks.
"""

import sys
from contextlib import ExitStack

import numpy as np

sys.path.insert(0, "/opt/trn_rl_repo")

import concourse.bass as bass  # noqa: E402
import concourse.tile as tile  # noqa: E402
from concourse import bacc  # noqa: E402
from concourse import mybir  # noqa: E402

F32 = mybir.dt.float32
BF16 = mybir.dt.bfloat16
AF = mybir.ActivationFunctionType

NEG = -10000.0
N_CORES = 8


def build_program(B=4, S=2048, io_dt=BF16):
    P = 128
    E = 1024
    ET = E // P            # 8 E-tiles
    RC = 512               # row chunk for projections
    NCH = S // RC          # proj chunks per batch (4)
    NT = S // P            # s-tiles per batch (16)
    TJ = S // 512          # query chunks of 512 per batch (4)
    ROWS = B * S

    nc = bacc.Bacc("TRN2", target_bir_lowering=False, debug=False)

    xT_d = nc.declare_dram_parameter("xT", [E, ROWS], io_dt, isOutput=False)
    wq_d = nc.declare_dram_parameter("wq", [E, P], io_dt, isOutput=False)
    wk_d = nc.declare_dram_parameter("wk", [E, P], io_dt, isOutput=False)
    wv_d = nc.declare_dram_parameter("wv", [E, P], io_dt, isOutput=False)
    bq_d = nc.declare_dram_parameter("bq", [P, 1], F32, isOutput=False)
    bv_d = nc.declare_dram_parameter("bv", [P, 1], F32, isOutput=False)
    ow_d = nc.declare_dram_parameter("ow", [P, E], io_dt, isOutput=False)
    msk_d = nc.declare_dram_parameter("msk", [P, B * NT], F32, isOutput=False)
    tri2_d = nc.declare_dram_parameter("tri2", [P, 2 * P], io_dt, isOutput=False)
    idn_d = nc.declare_dram_parameter("idn", [P, P], io_dt, isOutput=False)
    out_d = nc.declare_dram_parameter("outp", [ROWS, E], io_dt, isOutput=True)

    with ExitStack() as ctx:
        tc = ctx.enter_context(tile.TileContext(nc))
        const = ctx.enter_context(tc.tile_pool(name="const", bufs=1))

        wq_sb = [const.tile([P, P], io_dt, tag=f"wq{et}", name=f"wq{et}")
                 for et in range(ET)]
        wk_sb = [const.tile([P, P], io_dt, tag=f"wk{et}", name=f"wk{et}")
                 for et in range(ET)]
        wv_sb = [const.tile([P, P], io_dt, tag=f"wv{et}", name=f"wv{et}")
                 for et in range(ET)]
        for et in range(ET):
            esl = slice(et * P, (et + 1) * P)
            nc.gpsimd.dma_start(wq_sb[et][:], wq_d[esl, :])
            nc.scalar.dma_start(wk_sb[et][:], wk_d[esl, :])
            (nc.gpsimd if et % 2 else nc.scalar).dma_start(wv_sb[et][:], wv_d[esl, :])
        ow_sb = const.tile([P, E], io_dt, tag="ow")
        nc.scalar.dma_start(ow_sb[:], ow_d[:])
        bq_sb = const.tile([P, 1], F32, tag="bq")
        nc.gpsimd.dma_start(bq_sb[:], bq_d[:])
        bv_sb = const.tile([P, 1], F32, tag="bv")
        nc.gpsimd.dma_start(bv_sb[:], bv_d[:])
        msk_sb = const.tile([P, B * NT], F32, tag="msk")
        nc.gpsimd.dma_start(msk_sb[:], msk_d[:])
        tri2_sb = const.tile([P, 2 * P], io_dt, tag="tri2")
        nc.gpsimd.dma_start(tri2_sb[:], tri2_d[:])
        idn_sb = const.tile([P, P], io_dt, tag="idn")
        nc.gpsimd.dma_start(idn_sb[:], idn_d[:])
        ones_sb = const.tile([1, P], io_dt, tag="ones")
        nc.any.memset(ones_sb[:], 1.0)

        # per-batch projection outputs: q/k packed in one tile
        qkt_sbs = [const.tile([P, 2, S], io_dt, tag=f"qk{b}", name=f"qk{b}")
                   for b in range(B)]
        v_sbs = [const.tile([P, NT, 2, 65], io_dt, tag=f"v{b}", name=f"v{b}")
                 for b in range(B)]
        # ones-augmentation columns hold the key-padding mask (1 valid / 0
        # padded) so denominators count only valid keys
        for b in range(B):
            for h in range(2):
                nc.gpsimd.dma_start(v_sbs[b][:, :, h, 64:65],
                                    msk_d[:, b * NT:(b + 1) * NT])

        xpool = ctx.enter_context(tc.tile_pool(name="xp", bufs=2))
        ppool = ctx.enter_context(tc.tile_pool(name="pt", bufs=4))
        cpool = ctx.enter_context(tc.tile_pool(name="cn", bufs=2))
        spool = ctx.enter_context(tc.tile_pool(name="sm", bufs=2))
        opool = ctx.enter_context(tc.tile_pool(name="ot", bufs=4))
        # PSUM: "s" scores [128,2,512] x2 = 4 banks; "c" ctx accum x2 = 2
        # banks; "w" 1-bank work tiles x2 = 2 banks.  Total 8.
        psS = ctx.enter_context(tc.tile_pool(name="psS", bufs=2, space="PSUM"))
        psC = ctx.enter_context(tc.tile_pool(name="psC", bufs=2, space="PSUM"))
        psW = ctx.enter_context(tc.tile_pool(name="psW", bufs=2, space="PSUM"))

        def emit_x_dma(pc):
            """Prefetch x tiles for proj chunk pc (global index)."""
            b, ch = divmod(pc, NCH)
            r0 = b * S + ch * RC
            xt = xpool.tile([P, ET, RC], io_dt, tag="xt", name="xt")
            for et in range(ET):
                nc.sync.dma_start(xt[:, et], xT_d[et * P:(et + 1) * P, r0:r0 + RC])
            return xt

        def proj_units(pc, xt):
            """Yield filler closures for proj chunk pc. Each closure issues
            one PE matmul (plus any attached drain ops)."""
            b, ch = divmod(pc, NCH)
            rsb = slice(ch * RC, (ch + 1) * RC)
            acc = [None]
            vt = [None]

            def p_mm(w, et):
                # w: 0=q, 1=k, 2=v; each projection accumulates in its own
                # 1-bank work tile so the score-tile rotation is untouched
                if et == 0:
                    acc[0] = psW.tile([P, RC], F32, tag="w", name="pps")
                w_sb = (wq_sb, wk_sb, wv_sb)[w]
                nc.tensor.matmul(
                    acc[0][:], w_sb[et][:], xt[:, et],
                    start=(et == 0), stop=(et == ET - 1),
                )
                if et == ET - 1:
                    if w == 0:
                        nc.vector.tensor_scalar_add(
                            qkt_sbs[b][:, 0, rsb], acc[0][:], bq_sb[:])
                    elif w == 1:
                        # K bias is harmless (cancels in softmax) and
                        # omitted: plain downcast copy
                        nc.vector.tensor_copy(qkt_sbs[b][:, 1, rsb], acc[0][:])
                    else:
                        vt[0] = xpool.tile([P, RC], io_dt, tag="vt", name="vt")
                        nc.vector.tensor_scalar_add(
                            vt[0][:], acc[0][:], bv_sb[:])

            def t_mm(rt4):
                rt = ch * (RC // P) + rt4
                trp = psW.tile([P, 2, 64], io_dt, tag="w", name="trp")
                nc.tensor.transpose(
                    trp[:], vt[0][:, rt4 * P:(rt4 + 1) * P], idn_sb[:])
                msc = msk_sb[:, b * NT + rt:b * NT + rt + 1]
                # ACT reads PSUM at full rate; key-padding mask folded in as
                # the per-partition scale of a Copy activation
                nc.scalar.mul(v_sbs[b][:, rt, :, 0:64], trp[:], msc)

            for w in range(3):
                for et in range(ET):
                    yield lambda w=w, et=et: p_mm(w, et)
            for rt4 in range(RC // P):
                yield lambda rt4=rt4: t_mm(rt4)

        def outproj_units(b, j, cn):
            """Yield filler closures for the out-projection of chunk (b,j)."""
            t0 = j * 512

            def op_mm(rt4, fc):
                r0 = b * S + t0 + rt4 * P
                ops = psW.tile([P, 512], F32, tag="w", name="ops")
                nc.tensor.matmul(
                    ops[:],
                    cn[:, rt4 * P:(rt4 + 1) * P],
                    ow_sb[:, fc * 512:(fc + 1) * 512],
                    start=True, stop=True,
                )
                ot = opool.tile([P, 512], io_dt, tag="ot", name="ot")
                nc.vector.tensor_copy(ot[:], ops[:])
                nc.sync.dma_start(
                    out_d[r0:r0 + P, fc * 512:(fc + 1) * 512], ot[:])

            for rt4 in range(4):
                for fc in range(2):
                    yield lambda rt4=rt4, fc=fc: op_mm(rt4, fc)

        def attn_chunk(b, j, fillers):
            """Attention for query chunk (b,j), interleaving `fillers`
            (list of closures, each issuing one PE matmul) between
            iterations. Returns cn tile for the out-projection."""
            t0 = j * 512
            nv = 4 * j + 4
            cn = cpool.tile([P, 512], io_dt, tag="cn", name="cn")
            cps = [psC.tile([65, 512], F32, tag="c", name=f"cps{h}")
                   for h in range(2)]
            nf = len(fillers)
            fi = 0
            # reserve 2 fillers for the chunk-boundary normalization gap
            resv = min(2, nf)
            navail = nf - resv

            pend = [None]  # (sp2, pt2, i) awaiting ctx issue

            def issue_scores(i):
                delta = i * P - t0
                col0 = max(0, delta)
                sg = i * P
                sp2 = psS.tile([P, 2, 512], F32, tag="s", name="sp2")
                for h in range(2):
                    hp = slice(h * 64, (h + 1) * 64)
                    nc.tensor.matmul(
                        sp2[:, h, col0:512],
                        qkt_sbs[b][hp, 1, sg:sg + P],
                        qkt_sbs[b][hp, 0, t0 + col0:t0 + 512],
                        start=True, stop=(delta < 0),
                        skip_group_check=True,
                    )
                if delta >= 0:  # diagonal tile: one tri add for both heads
                    nc.tensor.matmul(
                        sp2[:, :, col0:col0 + P], idn_sb[:], tri2_sb[:],
                        start=False, stop=True, skip_group_check=True,
                    )
                pt2 = ppool.tile([P, 2, 512], io_dt, tag="pt", name="pt2")
                nc.scalar.activation(
                    pt2[:, :, col0:512], sp2[:, :, col0:512], AF.Exp)
                return (pt2, col0, i)

            def issue_ctx(pt2, col0, i):
                for h in range(2):
                    nc.tensor.matmul(
                        cps[h][:, col0:512],
                        v_sbs[b][:, i, h],
                        pt2[:, h, col0:512],
                        start=(i == 0), stop=(i == nv - 1),
                    )

            for i in range(nv):
                hi = (navail * (i + 1)) // nv
                while fi < hi:
                    fillers[fi]()
                    fi += 1
                nxt = issue_scores(i)
                if pend[0] is not None:
                    issue_ctx(*pend[0])
                pend[0] = nxt
            issue_ctx(*pend[0])

            # normalization: denom rows -> per-head PE broadcast into one
            # work tile -> one reciprocal -> per-head scale
            dens = []
            for h in range(2):
                den = spool.tile([1, 512], io_dt, tag="den", name="den")
                nc.vector.tensor_copy(den[:], cps[h][64:65, :])
                dens.append(den)
            while fi < nf:  # reserved fillers cover the DVE->PE gap
                fillers[fi]()
                fi += 1
            bps = psW.tile([P, 512], F32, tag="w", name="bps")
            for h in range(2):
                hp = slice(h * 64, (h + 1) * 64)
                nc.tensor.matmul(bps[hp, :], ones_sb[:, 0:64], dens[h][:],
                                 start=True, stop=True)
            rc = spool.tile([P, 512], F32, tag="rc", name="rc")
            nc.vector.reciprocal_approx_fast(rc[:], bps[:])
            for h in range(2):
                hp = slice(h * 64, (h + 1) * 64)
                nc.vector.tensor_mul(cn[hp, :], cps[h][0:64, :], rc[hp, :])
            return cn

        # ---- emission schedule ----
        # proj chunk pc runs as filler inside attn chunk pc-1 (deadline-JIT);
        # x tiles prefetch one attn chunk earlier.
        xts = {0: emit_x_dma(0), 1: emit_x_dma(1)}
        warm = list(proj_units(0, xts.pop(0)))
        for u in warm:
            u()
        prev = None  # (b, j, cn) of the chunk awaiting out-projection
        for b in range(B):
            for j in range(TJ):
                pc = 4 * b + j + 1
                if pc + 1 < B * NCH:
                    xts[pc + 1] = emit_x_dma(pc + 1)
                fillers = []
                if prev is not None:
                    fillers += list(outproj_units(*prev))
                if pc < B * NCH:
                    fillers += list(proj_units(pc, xts.pop(pc)))
                cn = attn_chunk(b, j, fillers)
                prev = (b, j, cn)
        for u in outproj_units(*prev):
            u()
    nc.compile()
    return nc


def make_core_inputs(x, key_padding_mask, Wqkv_w, Wqkv_b, out_w, B=4, S=2048,
                     np_io=None):
    """Host-side shard prep. Returns list of in_maps per core."""
    import ml_dtypes
    if np_io is None:
        np_io = ml_dtypes.bfloat16
    E = 1024
    P = 128
    NT = S // P
    x = np.asarray(x, np.float32)
    mask = np.asarray(key_padding_mask)
    Wqkv_w = np.asarray(Wqkv_w, np.float32)
    Wqkv_b = np.asarray(Wqkv_b, np.float32)
    out_w = np.asarray(out_w, np.float32)

    xT = np.ascontiguousarray(x.reshape(B * S, E).T).astype(np_io)
    m01 = mask.astype(np.float32)  # 1 valid / 0 padded
    msk_t = np.ascontiguousarray(m01.reshape(B * NT, P).T)  # [128, B*NT]
    r = np.arange(P)
    tri = np.where(r[:, None] > r[None, :], NEG, 0.0).astype(np.float32)
    tri2 = np.ascontiguousarray(np.concatenate([tri, tri], axis=1)).astype(np_io)
    idn = np.eye(P, dtype=np.float32).astype(np_io)
    scale = 1.0 / np.sqrt(64.0)

    in_maps = []
    for c in range(N_CORES):
        hA, hB = 2 * c, 2 * c + 1
        sel = np.r_[hA * 64:(hA + 1) * 64, hB * 64:(hB + 1) * 64]
        wq = np.ascontiguousarray(Wqkv_w[sel].T).astype(np_io)
        wk = np.ascontiguousarray((Wqkv_w[E + sel] * scale).T).astype(np_io)
        wv = np.ascontiguousarray(Wqkv_w[2 * E + sel].T).astype(np_io)
        bq = np.ascontiguousarray(Wqkv_b[sel][:, None]).astype(np.float32)
        bv = np.ascontiguousarray(Wqkv_b[2 * E + sel][:, None]).astype(np.float32)
        ow = np.ascontiguousarray(out_w[:, sel].T).astype(np_io)
        in_maps.append({
            "xT": xT, "wq": wq, "wk": wk, "wv": wv,
            "bq": bq, "bv": bv, "ow": ow, "msk": msk_t,
            "tri2": tri2, "idn": idn,
        })
    return in_maps


_NC_CACHE = {}


def _get_nc(B=4, S=2048, io_dt=BF16):
    key = (B, S, io_dt)
    if key not in _NC_CACHE:
        _NC_CACHE[key] = build_program(B, S, io_dt)
    return _NC_CACHE[key]


def run_full(inputs, trace=False, tmpdir=None, io_dt=BF16, np_io=None):
    from concourse.bass_utils import run_bass_kernel_spmd

    B, S, E = 4, 2048, 1024
    nc = _get_nc(B, S, io_dt)
    in_maps = make_core_inputs(
        inputs["x"], inputs["key_padding_mask"], inputs["Wqkv_w"],
        inputs["Wqkv_b"], inputs["out_w"], B, S, np_io=np_io,
    )
    res = run_bass_kernel_spmd(
        nc, in_maps, list(range(N_CORES)), trace=trace, tmpdir=tmpdir,
    )
    acc = res.results[0]["outp"].astype(np.float32)
    for c in range(1, N_CORES):
        acc = acc + res.results[c]["outp"].astype(np.float32)
    out = acc + np.asarray(inputs["out_b"], np.float32)[None, :]
    return out.reshape(B, S, E), res


def kernel(**inputs) -> np.ndarray:
    out, _ = run_full(inputs)
    return out
